# revision 4
# baseline (speedup 1.0000x reference)
# Tensor-parallel fused attention kernel for TRN2, 8 cores — v3.
# Core r owns heads {2r, 2r+1}. Inputs per core:
#   x     [2*S, 1024] f32  (replicated; rows = b*S + s)
#   wqkv  [1024, 512] f32  (columns: q_h0|q_h1|k_h0|k_h1|v_h0|v_h1|g_h0|g_h1,
#                           64 each; q/k columns de-interleaved on host:
#                           new[i]=old[2i], new[32+i]=old[2i+1])
#   wout  [1024, 1024] f32 (full, replicated — out-projection is token-sliced)
#   freqs [S, 32] f32      (replicated)
# Output per core:
#   out   [2*TPC, 1024] f32  (rows: batch0 tokens r*TPC.., then batch1)
#
# v3 structure: software-pipelined batch-steps.  During attention of batch s,
# the qkv+LN+rope phase of batch s+1 fills PE/DVE/ACT gaps, and the
# out-projection of batch s-1 (token-sliced after an AllToAll) fills more.
# AllToAll moves 8x fewer bytes than the v1/v2 AllGather.  Softmax
# denominators ride the PV matmul as a ones-column on v.  Single ACT table
# set (natural_log_exp).  Rope on host-de-interleaved contiguous blocks.
import math

import concourse.bass as bass
import concourse.mybir as mybir
from concourse import bacc, tile

F32 = mybir.dt.float32
F16 = mybir.dt.float16
AF = mybir.ActivationFunctionType
ALU = mybir.AluOpType
AX = mybir.AxisListType

DIM = 1024
HD = 64
EPS = 1e-5


def build(S: int, n_cores: int = 8, reps: int = 1, apply_ln_affine: bool = False):
    TB = S // 128            # token tiles per batch
    QW = min(512, S)         # queries per attention tile
    QT = S // QW             # q tiles per batch
    NQ = QW // 128           # token tiles per q tile
    TPC = S // n_cores       # tokens per core per batch (out-proj slice)
    NSTEP = 2 * reps
    TPQ = (TB + QT - 1) // QT  # p1 tiles emitted per attention q-tile

    nc = bacc.Bacc("TRN2", target_bir_lowering=False, debug=False,
                   num_devices=n_cores)

    X = nc.dram_tensor("x", [2 * S, DIM], F32, kind="ExternalInput")
    WQKV = nc.dram_tensor("wqkv", [DIM, 512], F32, kind="ExternalInput")
    WOUT = nc.dram_tensor("wout", [DIM, DIM], F32, kind="ExternalInput")
    FREQS = nc.dram_tensor("freqs", [S, 32], F32, kind="ExternalInput")
    LNP = nc.dram_tensor("lnp", [8, HD], F32, kind="ExternalInput")
    OUT = nc.dram_tensor("out", [2 * TPC, DIM], F32, kind="ExternalOutput")

    ag_in = [nc.dram_tensor(f"ag_in{p}", [n_cores * 128, TPC], F16)
             for p in range(2)]
    ag_out = [nc.dram_tensor(f"ag_out{p}", [n_cores * 128, TPC], F16)
              for p in range(2)]

    with tile.TileContext(nc) as tc:
        with (
            tc.tile_pool(name="persist", bufs=1) as pp,
            tc.tile_pool(name="work", bufs=2) as wp,
            tc.tile_pool(name="espool", bufs=4) as ep,
            tc.tile_pool(name="xstage", bufs=2) as xp,
            tc.tile_pool(name="ogf", bufs=1) as fp,
            tc.tile_pool(name="small", bufs=2) as sp,
        ):
            # ---- constants & weights ----
            w16 = pp.tile([128, 8, 512], F16, tag="w16")
            w16of = pp.tile([128, 8, DIM], F16, tag="w16of")
            for c in range(8):
                w32 = wp.tile([128, 512], F32, tag="wld")
                nc.gpsimd.dma_start(w32[:], WQKV[c * 128:(c + 1) * 128, :])
                nc.vector.tensor_copy(w16[:, c, :], w32[:])
                for o in range(2):
                    wo32 = wp.tile([128, 512], F32, tag="wld")
                    nc.gpsimd.dma_start(
                        wo32[:], WOUT[c * 128:(c + 1) * 128,
                                      o * 512:(o + 1) * 512])
                    nc.vector.tensor_copy(w16of[:, c, o * 512:(o + 1) * 512],
                                          wo32[:])

            if apply_ln_affine:
                lnp1 = sp.tile([1, 512], F32, tag="lnp1")
                nc.gpsimd.dma_start(
                    lnp1[:], LNP.ap().rearrange("a b -> (a b)").unsqueeze(0))
                ones1 = sp.tile([1, 128], F32, tag="ones1")
                nc.vector.memset(ones1[:], 1.0)
                with tc.tile_pool(name="pbc", bufs=1, space="PSUM") as pbc:
                    lnb_ps = pbc.tile([128, 512], F32)
                    nc.tensor.matmul(lnb_ps[:], ones1[:], lnp1[:], start=True,
                                     stop=True)
                    lnwb = pp.tile([128, 512], F16, tag="lnwb")
                    nc.vector.tensor_copy(lnwb[:], lnb_ps[:])

            # cos/sin tables, duplicated across both 32-halves of each 64
            ftile = sp.tile([128, TB * 32], F32, tag="ftile")
            nc.gpsimd.dma_start(
                ftile[:].rearrange("p (a c) -> p a c", c=32),
                bass.AP(FREQS.ap().tensor, 0, [[32, 128], [128 * 32, TB], [1, 32]]),
            )
            sin32 = sp.tile([128, TB * 32], F32, tag="sin32")
            cos32 = sp.tile([128, TB * 32], F32, tag="cos32")
            halfpi = sp.tile([128, 1], F32, tag="halfpi")
            nc.vector.memset(halfpi[:], math.pi / 2)
            epsc = pp.tile([128, 1], F32, tag="epsc")
            nc.vector.memset(epsc[:], EPS)
            nc.scalar.activation(sin32[:], ftile[:], AF.Sin)
            nc.scalar.activation(cos32[:], ftile[:], AF.Sin, bias=halfpi[:])
            c2t = pp.tile([128, TB, 64], F16, tag="c2t")
            s2t = pp.tile([128, TB, 64], F16, tag="s2t")
            c32v = cos32[:].rearrange("p (a c) -> p a c", c=32)
            s32v = sin32[:].rearrange("p (a c) -> p a c", c=32)
            nc.vector.tensor_copy(c2t[:, :, 0:32], c32v)
            nc.vector.tensor_copy(c2t[:, :, 32:64], c32v)
            nc.vector.tensor_copy(s2t[:, :, 0:32], s32v)
            nc.vector.tensor_copy(s2t[:, :, 32:64], s32v)

            onesP = pp.tile([128, 64], F16, tag="onesP")
            nc.vector.memset(onesP[:], 1.0)
            rdf = pp.tile([128, 2 * QW], F16, tag="rdf")

            # ---- per-parity activation buffers (pipeline depth 2) ----
            xT_s = pp.tile([128, TB, 8, 128], F16, tag="xTs")
            xTall = [xT_s, xT_s]
            v16, qkgT, gh1, og0, og1 = [], [], [], [], []
            for p in range(2):
                v16_p = pp.tile([128, TB, 130], F16, tag=f"v16{p}")
                nc.vector.memset(v16_p[:, :, 64:65], 1.0)
                nc.vector.memset(v16_p[:, :, 129:130], 1.0)
                v16.append(v16_p)
                qkgT_p = pp.tile([128, TB, 3, 128], F16, tag=f"qkgT{p}")
                qkgT.append(qkgT_p)
                gh1_p = pp.tile([64, TB, 128], F16, tag=f"gh1{p}")
                gh1.append(gh1_p)
                og0_p = pp.tile([64, S], F16, tag=f"og0{p}")
                og0.append(og0_p)
                og1_p = pp.tile([64, S], F16, tag=f"og1{p}")
                og1.append(og1_p)

            # ---- shared staging (live only within one step) ----
            qkg16 = pp.tile([128, TB, 384], F16, tag="qkg16")
            xc16 = pp.tile([128, TB, 256], F16, tag="xc16")
            negm = pp.tile([128, TB * 4], F32, tag="negm")
            rstd = pp.tile([128, TB * 4], F32, tag="rstd")
            t16a = pp.tile([128, TB * 256], F16, tag="t16a")
            t16b = pp.tile([128, TB * 256], F16, tag="t16b")

            with (
                tc.tile_pool(name="ps_s", bufs=2, space="PSUM") as ps_s,
                tc.tile_pool(name="po0", bufs=1, space="PSUM") as pop0,
                tc.tile_pool(name="po1", bufs=1, space="PSUM") as pop1,
                tc.tile_pool(name="p1q", bufs=2, space="PSUM") as p1q,
            ):

                def x_stage(b, par, T0, T1):
                    """batched x load + transpose, 4 tiles per DMA pair."""
                    Tg = T0
                    while Tg < T1:
                        g = min(4, T1 - Tg)
                        xt16 = xp.tile([128, 4 * DIM], F16, tag="xt16")
                        xv = xt16[:, 0:g * DIM]
                        r0 = (b * TB + Tg) * 128
                        nc.gpsimd.dma_start(
                            xv.rearrange("p (j t) -> p j t", t=DIM),
                            bass.AP(X.ap().tensor, r0 * DIM,
                                    [[DIM, 128], [128 * DIM, g], [1, DIM]]))
                        nc.sync.dma_start_transpose(
                            xTall[par][:, Tg:Tg + g, :, :], xv)
                        Tg += g

                def p1_tile(b, par, T):
                    """qkv matmul + evac for one tile (x already staged)."""
                    psq = p1q.tile([128, 512], F32, tag="psq")
                    for c in range(8):
                        nc.tensor.matmul(psq[:], xTall[par][:, T, c, :],
                                         w16[:, c, :],
                                         start=(c == 0), stop=(c == 7))
                    nm = negm[:, T * 4:(T + 1) * 4]
                    qk_ps = psq[:, 0:256].rearrange("p (a b) -> p a b", b=HD)
                    nc.vector.tensor_reduce(nm, qk_ps, AX.X, ALU.add)
                    nc.vector.tensor_scalar_mul(nm, nm, -1.0 / HD)
                    nc.vector.tensor_tensor(
                        xc16[:, T, :].rearrange("p (a b) -> p a b", b=HD),
                        qk_ps,
                        nm.unsqueeze(2).broadcast_to([128, 4, HD]), ALU.add)
                    v = v16[par]
                    nc.vector.tensor_copy(
                        bass.AP(v.tensor, v[:, T, 0].offset,
                                [[v[:].ap[0][0], 128], [65, 2], [1, HD]]),
                        psq[:, 256:384].rearrange("p (a b) -> p a b", b=HD))
                    nc.scalar.activation(qkg16[:, T, 256:384], psq[:, 384:512],
                                         AF.Exp, scale=-1.0)

                def p1_batch_range(b, par, T0, T1):
                    """LN rstd + scale + rope + sigmoid + transposes."""
                    H = T1 - T0
                    xch = xc16[:, T0:T0 + H, :]
                    xcf = xch.rearrange("p a b -> p (a b)")
                    th_a = t16a[:, T0 * 256:(T0 + H) * 256]
                    nc.vector.tensor_tensor(th_a, xcf, xcf, ALU.mult)
                    ssq = rstd[:, T0 * 4:(T0 + H) * 4]
                    nc.vector.tensor_reduce(
                        ssq, th_a.rearrange("p (a b) -> p a b", b=HD),
                        AX.X, ALU.add)
                    nc.scalar.activation(ssq, ssq, AF.Ln, bias=epsc[:],
                                         scale=1.0 / HD)
                    nc.scalar.activation(ssq, ssq, AF.Exp, scale=-0.5)
                    xcv = xch.rearrange("p a (s d) -> p (a s) d", d=HD)
                    nc.vector.tensor_tensor(
                        xcv, xcv,
                        ssq.unsqueeze(2).broadcast_to([128, H * 4, HD]),
                        ALU.mult)
                    if apply_ln_affine:
                        nc.vector.tensor_tensor(
                            xch, xch,
                            lnwb[:, 0:256].unsqueeze(1).broadcast_to(
                                [128, H, 256]), ALU.mult)
                        nc.vector.tensor_tensor(
                            xch, xch,
                            lnwb[:, 256:512].unsqueeze(1).broadcast_to(
                                [128, H, 256]), ALU.add)
                    # rope: ta = x'*c2, tb = x'*s2;
                    # r1 = ta[0:32]-tb[32:64]; r2 = tb[0:32]+ta[32:64]
                    xc4 = xch.rearrange("p a (s d) -> p a s d", d=HD)
                    cstep = c2t[:].ap[0][0]
                    c2b = bass.AP(c2t.tensor, c2t[:, T0, 0].offset,
                                  [[cstep, 128], [64, H], [0, 4], [1, 64]])
                    s2b = bass.AP(s2t.tensor, s2t[:, T0, 0].offset,
                                  [[cstep, 128], [64, H], [0, 4], [1, 64]])
                    ta = t16a[:, T0 * 256:(T0 + H) * 256].rearrange(
                        "p (a s d) -> p a s d", s=4, d=HD)
                    tb = t16b[:, T0 * 256:(T0 + H) * 256].rearrange(
                        "p (a s d) -> p a s d", s=4, d=HD)
                    nc.vector.tensor_tensor(ta, xc4, c2b, ALU.mult)
                    nc.vector.tensor_tensor(tb, xc4, s2b, ALU.mult)
                    qkv4 = qkg16[:, T0:T0 + H, 0:256].rearrange(
                        "p a (s d) -> p a s d", d=HD)
                    nc.vector.tensor_tensor(
                        qkv4[:, :, :, 0:32], ta[:, :, :, 0:32],
                        tb[:, :, :, 32:64], ALU.subtract)
                    nc.vector.tensor_tensor(
                        qkv4[:, :, :, 32:64], tb[:, :, :, 0:32],
                        ta[:, :, :, 32:64], ALU.add)
                    # sigmoid = 1/(1+exp(-g)); exp(-g) already staged
                    gview = qkg16[:, T0:T0 + H, 256:384]
                    tg = t16b[:, T0 * 128:(T0 + H) * 128].rearrange(
                        "p (a c) -> p a c", c=128)
                    nc.vector.tensor_scalar_add(tg, gview, 1.0)
                    with nc.allow_low_precision(reason="f16 sigmoid ok"):
                        nc.vector.reciprocal(gview, tg)
                    Tg = T0
                    while Tg < T0 + H:
                        g = min(4, T0 + H - Tg)
                        nc.sync.dma_start_transpose(
                            qkgT[par][:, Tg:Tg + g, :, :],
                            qkg16[:, Tg:Tg + g, :])
                        Tg += g
                    # shift gate h1 (partitions 64:128) down to 0:64
                    nc.gpsimd.dma_start(gh1[par][:, T0:T0 + H, :],
                                        qkgT[par][64:128, T0:T0 + H, 2, :])

                def attn_jloop(b, par, Q, po01):
                    qs0 = qkgT[par][0:64, Q * NQ:(Q + 1) * NQ, 0, :]
                    qs1 = qkgT[par][64:128, Q * NQ:(Q + 1) * NQ, 0, :]
                    po0, po1 = po01
                    for J in range(TB):
                        ps = ps_s.tile([128, 2 * QW], F32, tag="ps")
                        nc.tensor.matmul(ps[:, 0:QW], qkgT[par][0:64, J, 1, :],
                                         qs0, start=True, stop=True)
                        nc.tensor.matmul(ps[:, QW:2 * QW],
                                         qkgT[par][64:128, J, 1, :],
                                         qs1, start=True, stop=True)
                        es = ep.tile([128, 2 * QW], F16, tag="es")
                        nc.scalar.activation(es[:], ps[:], AF.Exp, scale=0.125)
                        nc.tensor.matmul(po0[:], v16[par][:, J, 0:65],
                                         es[:, 0:QW], start=(J == 0),
                                         stop=(J == TB - 1),
                                         skip_group_check=True)
                        nc.tensor.matmul(po1[:], v16[par][:, J, 65:130],
                                         es[:, QW:2 * QW], start=(J == 0),
                                         stop=(J == TB - 1),
                                         skip_group_check=True)

                def attn_finish(b, par, Q, po01):
                    po0, po1 = po01
                    with nc.allow_low_precision(reason="f16 recip-den"):
                        nc.vector.reciprocal(rdf[64:65, 0:QW], po0[64:65, :])
                        nc.vector.reciprocal(rdf[64:65, QW:2 * QW],
                                             po1[64:65, :])
                    prt = ps_s.tile([128, 2 * QW], F32, tag="ps")
                    pr0 = prt[0:64, 0:QW]
                    pr1 = prt[0:64, QW:2 * QW]
                    nc.tensor.matmul(pr0, onesP[64:65, :], rdf[64:65, 0:QW],
                                     start=True, stop=True)
                    nc.tensor.matmul(pr1, onesP[64:65, :],
                                     rdf[64:65, QW:2 * QW],
                                     start=True, stop=True)
                    gq0 = qkgT[par][0:64, Q * NQ:(Q + 1) * NQ, 2, :]
                    gq1 = gh1[par][:, Q * NQ:(Q + 1) * NQ, :]
                    u0 = wp.tile([64, QW], F32, tag="u")
                    u1 = wp.tile([64, QW], F32, tag="u")
                    nc.vector.tensor_tensor(
                        u0[:].rearrange("p (a t) -> p a t", t=128),
                        pr0.rearrange("p (a t) -> p a t", t=128), gq0, ALU.mult)
                    nc.vector.tensor_tensor(
                        u1[:].rearrange("p (a t) -> p a t", t=128),
                        pr1.rearrange("p (a t) -> p a t", t=128), gq1, ALU.mult)
                    qoff = Q * QW
                    nc.vector.tensor_tensor(og0[par][:, qoff:qoff + QW],
                                            po0[0:64, :], u0[:], ALU.mult)
                    nc.vector.tensor_tensor(og1[par][:, qoff:qoff + QW],
                                            po1[0:64, :], u1[:], ALU.mult)

                def attn_q(b, par, Q):
                    qs0 = qkgT[par][0:64, Q * NQ:(Q + 1) * NQ, 0, :]
                    qs1 = qkgT[par][64:128, Q * NQ:(Q + 1) * NQ, 0, :]
                    po0 = pop0.tile([65, QW], F32, tag="po0")
                    po1 = pop1.tile([65, QW], F32, tag="po1")
                    for J in range(TB):
                        ps = ps_s.tile([128, 2 * QW], F32, tag="ps")
                        nc.tensor.matmul(ps[:, 0:QW], qkgT[par][0:64, J, 1, :],
                                         qs0, start=True, stop=True)
                        nc.tensor.matmul(ps[:, QW:2 * QW],
                                         qkgT[par][64:128, J, 1, :],
                                         qs1, start=True, stop=True)
                        es = ep.tile([128, 2 * QW], F16, tag="es")
                        nc.scalar.activation(es[:], ps[:], AF.Exp, scale=0.125)
                        nc.tensor.matmul(po0[:], v16[par][:, J, 0:65],
                                         es[:, 0:QW], start=(J == 0),
                                         stop=(J == TB - 1),
                                         skip_group_check=True)
                        nc.tensor.matmul(po1[:], v16[par][:, J, 65:130],
                                         es[:, QW:2 * QW], start=(J == 0),
                                         stop=(J == TB - 1),
                                         skip_group_check=True)
                    with nc.allow_low_precision(reason="f16 recip-den"):
                        nc.vector.reciprocal(rdf[64:65, 0:QW], po0[64:65, :])
                        nc.vector.reciprocal(rdf[64:65, QW:2 * QW],
                                             po1[64:65, :])
                    prt = ps_s.tile([128, 2 * QW], F32, tag="ps")
                    pr0 = prt[0:64, 0:QW]
                    pr1 = prt[0:64, QW:2 * QW]
                    nc.tensor.matmul(pr0, onesP[64:65, :], rdf[64:65, 0:QW],
                                     start=True, stop=True)
                    nc.tensor.matmul(pr1, onesP[64:65, :],
                                     rdf[64:65, QW:2 * QW],
                                     start=True, stop=True)
                    gq0 = qkgT[par][0:64, Q * NQ:(Q + 1) * NQ, 2, :]
                    gq1 = gh1[par][:, Q * NQ:(Q + 1) * NQ, :]
                    u0 = wp.tile([64, QW], F32, tag="u")
                    u1 = wp.tile([64, QW], F32, tag="u")
                    nc.vector.tensor_tensor(
                        u0[:].rearrange("p (a t) -> p a t", t=128),
                        pr0.rearrange("p (a t) -> p a t", t=128), gq0, ALU.mult)
                    nc.vector.tensor_tensor(
                        u1[:].rearrange("p (a t) -> p a t", t=128),
                        pr1.rearrange("p (a t) -> p a t", t=128), gq1, ALU.mult)
                    qoff = Q * QW
                    nc.vector.tensor_tensor(og0[par][:, qoff:qoff + QW],
                                            po0[0:64, :], u0[:], ALU.mult)
                    nc.vector.tensor_tensor(og1[par][:, qoff:qoff + QW],
                                            po1[0:64, :], u1[:], ALU.mult)

                def emit_a2a(par):
                    # ag_in rows j*128+p <- og{0,1}[p, j*TPC + t]
                    ogt = og0[par]
                    nc.gpsimd.dma_start(
                        bass.AP(ag_in[par].ap().tensor, 0,
                                [[TPC, 64], [128 * TPC, n_cores], [1, TPC]]),
                        ogt[:].rearrange("p (j t) -> p j t", t=TPC))
                    ogt = og1[par]
                    nc.gpsimd.dma_start(
                        bass.AP(ag_in[par].ap().tensor, 64 * TPC,
                                [[TPC, 64], [128 * TPC, n_cores], [1, TPC]]),
                        ogt[:].rearrange("p (j t) -> p j t", t=TPC))
                    nc.gpsimd.collective_compute(
                        "AllToAll", ALU.bypass,
                        replica_groups=[list(range(n_cores))],
                        ins=[ag_in[par].ap()], outs=[ag_out[par].ap()],
                    )

                def outproj_load(par):
                    ogf = fp.tile([128, 8, TPC], F16, tag="ogf")
                    nc.sync.dma_start(
                        ogf[:],
                        bass.AP(ag_out[par].ap().tensor, 0,
                                [[TPC, 128], [128 * TPC, n_cores], [1, TPC]]))
                    return [ogf[:, c, :] for c in range(8)]

                TW = min(128, TPC)       # tokens per out-proj tile
                NOT = TPC // TW          # out-proj tiles per step

                def outproj_mm(b, tiles, i):
                    # token tile i (TW tokens) -> OUT rows b*TPC + i*TW
                    for o in range(DIM // 512):
                        pot = p1q.tile([128, 512], F32, tag="psq")
                        potv = pot[0:TW, :]
                        for c in range(8):
                            nc.tensor.matmul(
                                potv, tiles[c][:, i * TW:(i + 1) * TW],
                                w16of[:, c, o * 512:(o + 1) * 512],
                                start=(c == 0), stop=(c == 7))
                        ot32 = wp.tile([128, 512], F32, tag="ot32")
                        nc.vector.tensor_copy(ot32[0:TW, :], potv)
                        nc.gpsimd.dma_start(
                            OUT[b * TPC + i * TW: b * TPC + (i + 1) * TW,
                                o * 512:(o + 1) * 512], ot32[0:TW, :])

                # ==== prologue: full P1 for batch-step 0 ====
                x_stage(0, 0, 0, TB)
                for T in range(TB):
                    p1_tile(0, 0, T)
                p1_batch_range(0, 0, 0, TB // 2)
                p1_batch_range(0, 0, TB // 2, TB)

                # ==== steady-state steps ====
                prev_tiles = None
                for s in range(NSTEP):
                    b, par = s % 2, s % 2
                    nb, npar = (s + 1) % 2, (s + 1) % 2
                    has_next = s + 1 < NSTEP
                    # transposes serialize with in-flight collectives, so
                    # none of batch s+1's transposes may be scheduled while
                    # A2A(s-1) runs (the first ~30us of this step): p1 tiles
                    # go at the end of Q1 and right before the last q-tile.
                    t1 = 5 * TB // 8
                    for Q in range(QT):
                        po0_t = pop0.tile([65, QW], F32, tag="po0")
                        po1_t = pop1.tile([65, QW], F32, tag="po1")
                        po01 = (po0_t, po1_t)
                        attn_jloop(b, par, Q, po01)
                        if has_next and QT >= 3:
                            if Q == 1:
                                x_stage(nb, npar, 0, t1)
                                for T in range(0, t1):
                                    p1_tile(nb, npar, T)
                            elif Q == QT - 2:
                                x_stage(nb, npar, t1, TB)
                                for T in range(t1, TB):
                                    p1_tile(nb, npar, T)
                        attn_finish(b, par, Q, po01)
                        if has_next and QT >= 3:
                            if Q == 1:
                                p1_batch_range(nb, npar, 0, t1)
                            elif Q == QT - 2:
                                p1_batch_range(nb, npar, t1, TB)
                        if s >= 1 and QT >= 3 and Q == QT - 2:
                            prev_tiles = outproj_load(1 - par)
                    if has_next and QT < 3:
                        x_stage(nb, npar, 0, TB)
                        for T in range(TB):
                            p1_tile(nb, npar, T)
                        p1_batch_range(nb, npar, 0, TB)
                    if s >= 1 and QT < 3:
                        prev_tiles = outproj_load(1 - par)
                    if s >= 1:
                        for i in range(NOT):
                            outproj_mm(1 - b, prev_tiles, i)
                    emit_a2a(par)

                # ==== epilogue: out-projection of the last step ====
                last_par = (NSTEP - 1) % 2
                tiles = outproj_load(last_par)
                for i in range(NOT):
                    outproj_mm((NSTEP - 1) % 2, tiles, i)

    nc.compile()
    return nc


def _deinterleave_cols(w):
    # [..., 64] -> new[i]=old[2i], new[32+i]=old[2i+1]
    import numpy as np
    return np.concatenate([w[..., 0::2], w[..., 1::2]], axis=-1)


def shard_inputs(x, freqs, w_qkv, w_out, qn_w, qn_b, kn_w, kn_b, n_cores=8):
    import numpy as np
    B, S, _ = x.shape
    x2 = np.ascontiguousarray(x.reshape(2 * S, DIM), dtype=np.float32)
    qw = _deinterleave_cols(np.asarray(qn_w, np.float32))
    qb = _deinterleave_cols(np.asarray(qn_b, np.float32))
    kw = _deinterleave_cols(np.asarray(kn_w, np.float32))
    kb = _deinterleave_cols(np.asarray(kn_b, np.float32))
    lnp = np.stack([qw, qw, kw, kw, qb, qb, kb, kb]).astype(np.float32)
    wof = np.ascontiguousarray(w_out, dtype=np.float32)
    maps = []
    for r in range(n_cores):
        cols = []
        for sec in range(4):
            for h in range(2):
                c0 = sec * DIM + HD * (2 * r + h)
                blk = w_qkv[:, c0:c0 + HD]
                if sec < 2:  # q, k: de-interleave feature columns
                    blk = _deinterleave_cols(blk)
                cols.append(blk)
        wq = np.ascontiguousarray(np.concatenate(cols, axis=1), dtype=np.float32)
        maps.append({
            "x": x2, "wqkv": wq, "wout": wof,
            "freqs": np.ascontiguousarray(freqs, dtype=np.float32),
            "lnp": lnp,
        })
    return maps


def unshard_output(results, S):
    import numpy as np
    n = len(results)
    TPC = S // n
    out = np.empty((2, S, DIM), np.float32)
    for r in range(n):
        o = results[r]["out"]
        out[0, r * TPC:(r + 1) * TPC] = o[0:TPC]
        out[1, r * TPC:(r + 1) * TPC] = o[TPC:2 * TPC]
    return out


_NC_CACHE = {}


def _get_nc(S, affine):
    key = (S, affine)
    if key not in _NC_CACHE:
        _NC_CACHE[key] = build(S, apply_ln_affine=affine)
    return _NC_CACHE[key]


def kernel(x, freqs, w_qkv, w_out, qn_w, qn_b, kn_w, kn_b):
    """Full-input entrypoint: shards across 8 neuron cores, runs, gathers."""
    import numpy as np
    from concourse.bass_utils import run_bass_kernel_spmd

    x = np.asarray(x, dtype=np.float32)
    freqs = np.asarray(freqs, dtype=np.float32)
    w_qkv = np.asarray(w_qkv, dtype=np.float32)
    w_out = np.asarray(w_out, dtype=np.float32)
    qn_w, qn_b = np.asarray(qn_w), np.asarray(qn_b)
    kn_w, kn_b = np.asarray(kn_w), np.asarray(kn_b)
    B, S, _ = x.shape
    affine = not (np.all(qn_w == 1) and np.all(qn_b == 0)
                  and np.all(kn_w == 1) and np.all(kn_b == 0))
    nc = _get_nc(S, bool(affine))
    maps = shard_inputs(x, freqs, w_qkv, w_out, qn_w, qn_b, kn_w, kn_b)
    res = run_bass_kernel_spmd(nc, maps, list(range(8)))
    return unshard_output(res.results, S)


# revision 5
# speedup vs baseline: 1.1940x; 1.1940x over previous
# Tensor-parallel fused attention kernel for TRN2, 8 cores — v3.
# Core r owns heads {2r, 2r+1}. Inputs per core:
#   x     [2*S, 1024] f32  (replicated; rows = b*S + s)
#   wqkv  [1024, 512] f32  (columns: q_h0|q_h1|k_h0|k_h1|v_h0|v_h1|g_h0|g_h1,
#                           64 each; q/k columns de-interleaved on host:
#                           new[i]=old[2i], new[32+i]=old[2i+1])
#   wout  [1024, 1024] f32 (full, replicated — out-projection is token-sliced)
#   freqs [S, 32] f32      (replicated)
# Output per core:
#   out   [2*TPC, 1024] f32  (rows: batch0 tokens r*TPC.., then batch1)
#
# v3 structure: software-pipelined batch-steps.  During attention of batch s,
# the qkv+LN+rope phase of batch s+1 fills PE/DVE/ACT gaps, and the
# out-projection of batch s-1 (token-sliced after an AllToAll) fills more.
# AllToAll moves 8x fewer bytes than the v1/v2 AllGather.  Softmax
# denominators ride the PV matmul as a ones-column on v.  Single ACT table
# set (natural_log_exp).  Rope on host-de-interleaved contiguous blocks.
import math

import concourse.bass as bass
import concourse.mybir as mybir
from concourse import bacc, tile

F32 = mybir.dt.float32
F16 = mybir.dt.float16
AF = mybir.ActivationFunctionType
ALU = mybir.AluOpType
AX = mybir.AxisListType

DIM = 1024
HD = 64
EPS = 1e-5


def build(S: int, n_cores: int = 8, reps: int = 1, apply_ln_affine: bool = False):
    TB = S // 128            # token tiles per batch
    QW = min(512, S)         # queries per attention tile
    QT = S // QW             # q tiles per batch
    NQ = QW // 128           # token tiles per q tile
    TPC = S // n_cores       # tokens per core per batch (out-proj slice)
    NSTEP = 2 * reps
    TPQ = (TB + QT - 1) // QT  # p1 tiles emitted per attention q-tile

    nc = bacc.Bacc("TRN2", target_bir_lowering=False, debug=False,
                   num_devices=n_cores)

    X = nc.dram_tensor("x", [2 * S, DIM], F32, kind="ExternalInput")
    WQKV = nc.dram_tensor("wqkv", [DIM, 512], F32, kind="ExternalInput")
    WOUT = nc.dram_tensor("wout", [DIM, DIM], F32, kind="ExternalInput")
    FREQS = nc.dram_tensor("freqs", [S, 32], F32, kind="ExternalInput")
    LNP = nc.dram_tensor("lnp", [8, HD], F32, kind="ExternalInput")
    OUT = nc.dram_tensor("out", [2 * TPC, DIM], F32, kind="ExternalOutput")

    ag_in = [nc.dram_tensor(f"ag_in{p}", [n_cores * 128, TPC], F16)
             for p in range(2)]
    ag_out = [nc.dram_tensor(f"ag_out{p}", [n_cores * 128, TPC], F16)
              for p in range(2)]

    with tile.TileContext(nc) as tc:
        with (
            tc.tile_pool(name="persist", bufs=1) as pp,
            tc.tile_pool(name="work", bufs=2) as wp,
            tc.tile_pool(name="espool", bufs=4) as ep,
            tc.tile_pool(name="xstage", bufs=6) as xp,
            tc.tile_pool(name="ogf", bufs=10) as fp,
            tc.tile_pool(name="small", bufs=2) as sp,
        ):
            # ---- constants & weights ----
            w16 = pp.tile([128, 8, 512], F16, tag="w16")
            w16of = pp.tile([128, 8, DIM], F16, tag="w16of")
            for c in range(8):
                w32 = wp.tile([128, 512], F32, tag="wld")
                nc.gpsimd.dma_start(w32[:], WQKV[c * 128:(c + 1) * 128, :])
                nc.vector.tensor_copy(w16[:, c, :], w32[:])
                for o in range(2):
                    wo32 = wp.tile([128, 512], F32, tag="wld")
                    nc.gpsimd.dma_start(
                        wo32[:], WOUT[c * 128:(c + 1) * 128,
                                      o * 512:(o + 1) * 512])
                    nc.vector.tensor_copy(w16of[:, c, o * 512:(o + 1) * 512],
                                          wo32[:])

            if apply_ln_affine:
                lnp1 = sp.tile([1, 512], F32, tag="lnp1")
                nc.gpsimd.dma_start(
                    lnp1[:], LNP.ap().rearrange("a b -> (a b)").unsqueeze(0))
                ones1 = sp.tile([1, 128], F32, tag="ones1")
                nc.vector.memset(ones1[:], 1.0)
                with tc.tile_pool(name="pbc", bufs=1, space="PSUM") as pbc:
                    lnb_ps = pbc.tile([128, 512], F32)
                    nc.tensor.matmul(lnb_ps[:], ones1[:], lnp1[:], start=True,
                                     stop=True)
                    lnwb = pp.tile([128, 512], F16, tag="lnwb")
                    nc.vector.tensor_copy(lnwb[:], lnb_ps[:])

            # cos/sin tables, duplicated across both 32-halves of each 64
            ftile = sp.tile([128, TB * 32], F32, tag="ftile")
            nc.gpsimd.dma_start(
                ftile[:].rearrange("p (a c) -> p a c", c=32),
                bass.AP(FREQS.ap().tensor, 0, [[32, 128], [128 * 32, TB], [1, 32]]),
            )
            sin32 = sp.tile([128, TB * 32], F32, tag="sin32")
            cos32 = sp.tile([128, TB * 32], F32, tag="cos32")
            halfpi = sp.tile([128, 1], F32, tag="halfpi")
            nc.vector.memset(halfpi[:], math.pi / 2)
            epsc = pp.tile([128, 1], F32, tag="epsc")
            nc.vector.memset(epsc[:], EPS)
            nc.scalar.activation(sin32[:], ftile[:], AF.Sin)
            nc.scalar.activation(cos32[:], ftile[:], AF.Sin, bias=halfpi[:])
            c2t = pp.tile([128, TB, 64], F16, tag="c2t")
            s2t = pp.tile([128, TB, 64], F16, tag="s2t")
            c32v = cos32[:].rearrange("p (a c) -> p a c", c=32)
            s32v = sin32[:].rearrange("p (a c) -> p a c", c=32)
            nc.vector.tensor_copy(c2t[:, :, 0:32], c32v)
            nc.vector.tensor_copy(c2t[:, :, 32:64], c32v)
            nc.vector.tensor_copy(s2t[:, :, 0:32], s32v)
            nc.vector.tensor_copy(s2t[:, :, 32:64], s32v)

            onesP = pp.tile([128, 64], F16, tag="onesP")
            nc.vector.memset(onesP[:], 1.0)
            rdf = pp.tile([128, 2 * QW], F16, tag="rdf")

            # ---- per-parity activation buffers (pipeline depth 2) ----
            xT_s = pp.tile([128, TB, 8, 128], F16, tag="xTs")
            xTall = [xT_s, xT_s]
            v16, qkgT, gh1, og0, og1 = [], [], [], [], []
            for p in range(2):
                v16_p = pp.tile([128, TB, 130], F16, tag=f"v16{p}")
                nc.vector.memset(v16_p[:, :, 64:65], 1.0)
                nc.vector.memset(v16_p[:, :, 129:130], 1.0)
                v16.append(v16_p)
                qkgT_p = pp.tile([128, TB, 3, 128], F16, tag=f"qkgT{p}")
                qkgT.append(qkgT_p)
                gh1_p = pp.tile([64, TB, 128], F16, tag=f"gh1{p}")
                gh1.append(gh1_p)
                og0_p = pp.tile([64, S], F16, tag=f"og0{p}")
                og0.append(og0_p)
                og1_p = pp.tile([64, S], F16, tag=f"og1{p}")
                og1.append(og1_p)

            # ---- shared staging (live only within one step) ----
            qkg16 = pp.tile([128, TB, 384], F16, tag="qkg16")
            xc16 = pp.tile([128, TB, 256], F16, tag="xc16")
            negm = pp.tile([128, TB * 4], F32, tag="negm")
            rstd = pp.tile([128, TB * 4], F32, tag="rstd")
            t16a = pp.tile([128, TB * 256], F16, tag="t16a")
            t16b = pp.tile([128, TB * 256], F16, tag="t16b")

            with (
                tc.tile_pool(name="ps_s", bufs=2, space="PSUM") as ps_s,
                tc.tile_pool(name="po0", bufs=1, space="PSUM") as pop0,
                tc.tile_pool(name="po1", bufs=1, space="PSUM") as pop1,
                tc.tile_pool(name="p1q", bufs=2, space="PSUM") as p1q,
            ):

                def p1_tile(b, par, T):
                    """x-load + transpose + qkv matmul + evac for one tile."""
                    xt16 = xp.tile([128, DIM], F16, tag="xt16")
                    nc.gpsimd.dma_start(
                        xt16[:], X[(b * TB + T) * 128:(b * TB + T + 1) * 128, :])
                    nc.sync.dma_start_transpose(xTall[par][:, T, :, :], xt16[:])
                    psq = p1q.tile([128, 512], F32, tag="psq")
                    for c in range(8):
                        nc.tensor.matmul(psq[:], xTall[par][:, T, c, :],
                                         w16[:, c, :],
                                         start=(c == 0), stop=(c == 7))
                    nm = negm[:, T * 4:(T + 1) * 4]
                    qk_ps = psq[:, 0:256].rearrange("p (a b) -> p a b", b=HD)
                    nc.vector.tensor_reduce(nm, qk_ps, AX.X, ALU.add)
                    nc.vector.tensor_scalar_mul(nm, nm, -1.0 / HD)
                    nc.vector.tensor_tensor(
                        xc16[:, T, :].rearrange("p (a b) -> p a b", b=HD),
                        qk_ps,
                        nm.unsqueeze(2).broadcast_to([128, 4, HD]), ALU.add)
                    v = v16[par]
                    nc.vector.tensor_copy(
                        bass.AP(v.tensor, v[:, T, 0].offset,
                                [[v[:].ap[0][0], 128], [65, 2], [1, HD]]),
                        psq[:, 256:384].rearrange("p (a b) -> p a b", b=HD))
                    nc.scalar.activation(qkg16[:, T, 256:384], psq[:, 384:512],
                                         AF.Exp, scale=-1.0)

                def p1_batch_range(b, par, T0, T1):
                    """LN rstd + scale + rope + sigmoid + transposes."""
                    H = T1 - T0
                    xch = xc16[:, T0:T0 + H, :]
                    xcf = xch.rearrange("p a b -> p (a b)")
                    th_a = t16a[:, T0 * 256:(T0 + H) * 256]
                    nc.vector.tensor_tensor(th_a, xcf, xcf, ALU.mult)
                    ssq = rstd[:, T0 * 4:(T0 + H) * 4]
                    nc.vector.tensor_reduce(
                        ssq, th_a.rearrange("p (a b) -> p a b", b=HD),
                        AX.X, ALU.add)
                    nc.scalar.activation(ssq, ssq, AF.Ln, bias=epsc[:],
                                         scale=1.0 / HD)
                    nc.scalar.activation(ssq, ssq, AF.Exp, scale=-0.5)
                    xcv = xch.rearrange("p a (s d) -> p (a s) d", d=HD)
                    nc.vector.tensor_tensor(
                        xcv, xcv,
                        ssq.unsqueeze(2).broadcast_to([128, H * 4, HD]),
                        ALU.mult)
                    if apply_ln_affine:
                        nc.vector.tensor_tensor(
                            xch, xch,
                            lnwb[:, 0:256].unsqueeze(1).broadcast_to(
                                [128, H, 256]), ALU.mult)
                        nc.vector.tensor_tensor(
                            xch, xch,
                            lnwb[:, 256:512].unsqueeze(1).broadcast_to(
                                [128, H, 256]), ALU.add)
                    # rope: ta = x'*c2, tb = x'*s2;
                    # r1 = ta[0:32]-tb[32:64]; r2 = tb[0:32]+ta[32:64]
                    xc4 = xch.rearrange("p a (s d) -> p a s d", d=HD)
                    cstep = c2t[:].ap[0][0]
                    c2b = bass.AP(c2t.tensor, c2t[:, T0, 0].offset,
                                  [[cstep, 128], [64, H], [0, 4], [1, 64]])
                    s2b = bass.AP(s2t.tensor, s2t[:, T0, 0].offset,
                                  [[cstep, 128], [64, H], [0, 4], [1, 64]])
                    ta = t16a[:, T0 * 256:(T0 + H) * 256].rearrange(
                        "p (a s d) -> p a s d", s=4, d=HD)
                    tb = t16b[:, T0 * 256:(T0 + H) * 256].rearrange(
                        "p (a s d) -> p a s d", s=4, d=HD)
                    nc.vector.tensor_tensor(ta, xc4, c2b, ALU.mult)
                    nc.vector.tensor_tensor(tb, xc4, s2b, ALU.mult)
                    qkv4 = qkg16[:, T0:T0 + H, 0:256].rearrange(
                        "p a (s d) -> p a s d", d=HD)
                    nc.vector.tensor_tensor(
                        qkv4[:, :, :, 0:32], ta[:, :, :, 0:32],
                        tb[:, :, :, 32:64], ALU.subtract)
                    nc.vector.tensor_tensor(
                        qkv4[:, :, :, 32:64], tb[:, :, :, 0:32],
                        ta[:, :, :, 32:64], ALU.add)
                    # sigmoid = 1/(1+exp(-g)); exp(-g) already staged
                    gview = qkg16[:, T0:T0 + H, 256:384]
                    tg = t16b[:, T0 * 128:(T0 + H) * 128].rearrange(
                        "p (a c) -> p a c", c=128)
                    nc.vector.tensor_scalar_add(tg, gview, 1.0)
                    with nc.allow_low_precision(reason="f16 sigmoid ok"):
                        nc.vector.reciprocal(gview, tg)
                    Tg = T0
                    while Tg < T0 + H:
                        g = min(4, T0 + H - Tg)
                        nc.sync.dma_start_transpose(
                            qkgT[par][:, Tg:Tg + g, :, :],
                            qkg16[:, Tg:Tg + g, :])
                        Tg += g
                    # shift gate h1 (partitions 64:128) down to 0:64
                    nc.gpsimd.dma_start(gh1[par][:, T0:T0 + H, :],
                                        qkgT[par][64:128, T0:T0 + H, 2, :])

                def attn_jloop(b, par, Q, po01):
                    qs0 = qkgT[par][0:64, Q * NQ:(Q + 1) * NQ, 0, :]
                    qs1 = qkgT[par][64:128, Q * NQ:(Q + 1) * NQ, 0, :]
                    po0, po1 = po01
                    for J in range(TB):
                        ps = ps_s.tile([128, 2 * QW], F32, tag="ps")
                        nc.tensor.matmul(ps[:, 0:QW], qkgT[par][0:64, J, 1, :],
                                         qs0, start=True, stop=True)
                        nc.tensor.matmul(ps[:, QW:2 * QW],
                                         qkgT[par][64:128, J, 1, :],
                                         qs1, start=True, stop=True)
                        es = ep.tile([128, 2 * QW], F16, tag="es")
                        nc.scalar.activation(es[:], ps[:], AF.Exp, scale=0.125)
                        nc.tensor.matmul(po0[:], v16[par][:, J, 0:65],
                                         es[:, 0:QW], start=(J == 0),
                                         stop=(J == TB - 1),
                                         skip_group_check=True)
                        nc.tensor.matmul(po1[:], v16[par][:, J, 65:130],
                                         es[:, QW:2 * QW], start=(J == 0),
                                         stop=(J == TB - 1),
                                         skip_group_check=True)

                def attn_finish(b, par, Q, po01):
                    po0, po1 = po01
                    with nc.allow_low_precision(reason="f16 recip-den"):
                        nc.vector.reciprocal(rdf[64:65, 0:QW], po0[64:65, :])
                        nc.vector.reciprocal(rdf[64:65, QW:2 * QW],
                                             po1[64:65, :])
                    prt = ps_s.tile([128, 2 * QW], F32, tag="ps")
                    pr0 = prt[0:64, 0:QW]
                    pr1 = prt[0:64, QW:2 * QW]
                    nc.tensor.matmul(pr0, onesP[64:65, :], rdf[64:65, 0:QW],
                                     start=True, stop=True)
                    nc.tensor.matmul(pr1, onesP[64:65, :],
                                     rdf[64:65, QW:2 * QW],
                                     start=True, stop=True)
                    gq0 = qkgT[par][0:64, Q * NQ:(Q + 1) * NQ, 2, :]
                    gq1 = gh1[par][:, Q * NQ:(Q + 1) * NQ, :]
                    u0 = wp.tile([64, QW], F32, tag="u")
                    u1 = wp.tile([64, QW], F32, tag="u")
                    nc.vector.tensor_tensor(
                        u0[:].rearrange("p (a t) -> p a t", t=128),
                        pr0.rearrange("p (a t) -> p a t", t=128), gq0, ALU.mult)
                    nc.vector.tensor_tensor(
                        u1[:].rearrange("p (a t) -> p a t", t=128),
                        pr1.rearrange("p (a t) -> p a t", t=128), gq1, ALU.mult)
                    qoff = Q * QW
                    nc.vector.tensor_tensor(og0[par][:, qoff:qoff + QW],
                                            po0[0:64, :], u0[:], ALU.mult)
                    nc.vector.tensor_tensor(og1[par][:, qoff:qoff + QW],
                                            po1[0:64, :], u1[:], ALU.mult)

                def attn_q(b, par, Q):
                    qs0 = qkgT[par][0:64, Q * NQ:(Q + 1) * NQ, 0, :]
                    qs1 = qkgT[par][64:128, Q * NQ:(Q + 1) * NQ, 0, :]
                    po0 = pop0.tile([65, QW], F32, tag="po0")
                    po1 = pop1.tile([65, QW], F32, tag="po1")
                    for J in range(TB):
                        ps = ps_s.tile([128, 2 * QW], F32, tag="ps")
                        nc.tensor.matmul(ps[:, 0:QW], qkgT[par][0:64, J, 1, :],
                                         qs0, start=True, stop=True)
                        nc.tensor.matmul(ps[:, QW:2 * QW],
                                         qkgT[par][64:128, J, 1, :],
                                         qs1, start=True, stop=True)
                        es = ep.tile([128, 2 * QW], F16, tag="es")
                        nc.scalar.activation(es[:], ps[:], AF.Exp, scale=0.125)
                        nc.tensor.matmul(po0[:], v16[par][:, J, 0:65],
                                         es[:, 0:QW], start=(J == 0),
                                         stop=(J == TB - 1),
                                         skip_group_check=True)
                        nc.tensor.matmul(po1[:], v16[par][:, J, 65:130],
                                         es[:, QW:2 * QW], start=(J == 0),
                                         stop=(J == TB - 1),
                                         skip_group_check=True)
                    with nc.allow_low_precision(reason="f16 recip-den"):
                        nc.vector.reciprocal(rdf[64:65, 0:QW], po0[64:65, :])
                        nc.vector.reciprocal(rdf[64:65, QW:2 * QW],
                                             po1[64:65, :])
                    prt = ps_s.tile([128, 2 * QW], F32, tag="ps")
                    pr0 = prt[0:64, 0:QW]
                    pr1 = prt[0:64, QW:2 * QW]
                    nc.tensor.matmul(pr0, onesP[64:65, :], rdf[64:65, 0:QW],
                                     start=True, stop=True)
                    nc.tensor.matmul(pr1, onesP[64:65, :],
                                     rdf[64:65, QW:2 * QW],
                                     start=True, stop=True)
                    gq0 = qkgT[par][0:64, Q * NQ:(Q + 1) * NQ, 2, :]
                    gq1 = gh1[par][:, Q * NQ:(Q + 1) * NQ, :]
                    u0 = wp.tile([64, QW], F32, tag="u")
                    u1 = wp.tile([64, QW], F32, tag="u")
                    nc.vector.tensor_tensor(
                        u0[:].rearrange("p (a t) -> p a t", t=128),
                        pr0.rearrange("p (a t) -> p a t", t=128), gq0, ALU.mult)
                    nc.vector.tensor_tensor(
                        u1[:].rearrange("p (a t) -> p a t", t=128),
                        pr1.rearrange("p (a t) -> p a t", t=128), gq1, ALU.mult)
                    qoff = Q * QW
                    nc.vector.tensor_tensor(og0[par][:, qoff:qoff + QW],
                                            po0[0:64, :], u0[:], ALU.mult)
                    nc.vector.tensor_tensor(og1[par][:, qoff:qoff + QW],
                                            po1[0:64, :], u1[:], ALU.mult)

                def emit_a2a(par):
                    # ag_in rows j*128+p <- og{0,1}[p, j*TPC + t]
                    ogt = og0[par]
                    nc.gpsimd.dma_start(
                        bass.AP(ag_in[par].ap().tensor, 0,
                                [[TPC, 64], [128 * TPC, n_cores], [1, TPC]]),
                        ogt[:].rearrange("p (j t) -> p j t", t=TPC))
                    ogt = og1[par]
                    nc.gpsimd.dma_start(
                        bass.AP(ag_in[par].ap().tensor, 64 * TPC,
                                [[TPC, 64], [128 * TPC, n_cores], [1, TPC]]),
                        ogt[:].rearrange("p (j t) -> p j t", t=TPC))
                    nc.gpsimd.collective_compute(
                        "AllToAll", ALU.bypass,
                        replica_groups=[list(range(n_cores))],
                        ins=[ag_in[par].ap()], outs=[ag_out[par].ap()],
                    )

                def outproj_load(par):
                    tiles = []
                    for c in range(8):
                        ogf = fp.tile([128, TPC], F16, tag="ogf")
                        nc.sync.dma_start(
                            ogf[:], ag_out[par][c * 128:(c + 1) * 128, :])
                        tiles.append(ogf)
                    return tiles

                TW = min(128, TPC)       # tokens per out-proj tile
                NOT = TPC // TW          # out-proj tiles per step

                def outproj_mm(b, tiles, i):
                    # token tile i (TW tokens) -> OUT rows b*TPC + i*TW
                    for o in range(DIM // 512):
                        pot = p1q.tile([128, 512], F32, tag="psq")
                        potv = pot[0:TW, :]
                        for c in range(8):
                            nc.tensor.matmul(
                                potv, tiles[c][:, i * TW:(i + 1) * TW],
                                w16of[:, c, o * 512:(o + 1) * 512],
                                start=(c == 0), stop=(c == 7))
                        ot32 = wp.tile([128, 512], F32, tag="ot32")
                        nc.vector.tensor_copy(ot32[0:TW, :], potv)
                        nc.gpsimd.dma_start(
                            OUT[b * TPC + i * TW: b * TPC + (i + 1) * TW,
                                o * 512:(o + 1) * 512], ot32[0:TW, :])

                # ==== prologue: full P1 for batch-step 0 ====
                for T in range(TB):
                    p1_tile(0, 0, T)
                p1_batch_range(0, 0, 0, TB // 2)
                p1_batch_range(0, 0, TB // 2, TB)

                # ==== steady-state steps ====
                prev_tiles = None
                for s in range(NSTEP):
                    b, par = s % 2, s % 2
                    nb, npar = (s + 1) % 2, (s + 1) % 2
                    has_next = s + 1 < NSTEP
                    # transposes serialize with in-flight collectives, so
                    # none of batch s+1's transposes may be scheduled while
                    # A2A(s-1) runs (the first ~30us of this step): p1 tiles
                    # go at the end of Q1 and right before the last q-tile.
                    t1 = 5 * TB // 8
                    for Q in range(QT):
                        po0_t = pop0.tile([65, QW], F32, tag="po0")
                        po1_t = pop1.tile([65, QW], F32, tag="po1")
                        po01 = (po0_t, po1_t)
                        attn_jloop(b, par, Q, po01)
                        if has_next and QT >= 3:
                            if Q == 1:
                                for T in range(0, t1):
                                    p1_tile(nb, npar, T)
                            elif Q == QT - 2:
                                for T in range(t1, TB):
                                    p1_tile(nb, npar, T)
                        attn_finish(b, par, Q, po01)
                        if has_next and QT >= 3:
                            if Q == 1:
                                p1_batch_range(nb, npar, 0, t1)
                            elif Q == QT - 2:
                                p1_batch_range(nb, npar, t1, TB)
                        if s >= 1 and QT >= 3 and Q == QT - 2:
                            prev_tiles = outproj_load(1 - par)
                    if has_next and QT < 3:
                        for T in range(TB):
                            p1_tile(nb, npar, T)
                        p1_batch_range(nb, npar, 0, TB)
                    if s >= 1 and QT < 3:
                        prev_tiles = outproj_load(1 - par)
                    if s >= 1:
                        for i in range(NOT):
                            outproj_mm(1 - b, prev_tiles, i)
                    emit_a2a(par)

                # ==== epilogue: out-projection of the last step ====
                last_par = (NSTEP - 1) % 2
                tiles = outproj_load(last_par)
                for i in range(NOT):
                    outproj_mm((NSTEP - 1) % 2, tiles, i)

    nc.compile()
    return nc


def _deinterleave_cols(w):
    # [..., 64] -> new[i]=old[2i], new[32+i]=old[2i+1]
    import numpy as np
    return np.concatenate([w[..., 0::2], w[..., 1::2]], axis=-1)


def shard_inputs(x, freqs, w_qkv, w_out, qn_w, qn_b, kn_w, kn_b, n_cores=8):
    import numpy as np
    B, S, _ = x.shape
    x2 = np.ascontiguousarray(x.reshape(2 * S, DIM), dtype=np.float32)
    qw = _deinterleave_cols(np.asarray(qn_w, np.float32))
    qb = _deinterleave_cols(np.asarray(qn_b, np.float32))
    kw = _deinterleave_cols(np.asarray(kn_w, np.float32))
    kb = _deinterleave_cols(np.asarray(kn_b, np.float32))
    lnp = np.stack([qw, qw, kw, kw, qb, qb, kb, kb]).astype(np.float32)
    wof = np.ascontiguousarray(w_out, dtype=np.float32)
    maps = []
    for r in range(n_cores):
        cols = []
        for sec in range(4):
            for h in range(2):
                c0 = sec * DIM + HD * (2 * r + h)
                blk = w_qkv[:, c0:c0 + HD]
                if sec < 2:  # q, k: de-interleave feature columns
                    blk = _deinterleave_cols(blk)
                cols.append(blk)
        wq = np.ascontiguousarray(np.concatenate(cols, axis=1), dtype=np.float32)
        maps.append({
            "x": x2, "wqkv": wq, "wout": wof,
            "freqs": np.ascontiguousarray(freqs, dtype=np.float32),
            "lnp": lnp,
        })
    return maps


def unshard_output(results, S):
    import numpy as np
    n = len(results)
    TPC = S // n
    out = np.empty((2, S, DIM), np.float32)
    for r in range(n):
        o = results[r]["out"]
        out[0, r * TPC:(r + 1) * TPC] = o[0:TPC]
        out[1, r * TPC:(r + 1) * TPC] = o[TPC:2 * TPC]
    return out


_NC_CACHE = {}


def _get_nc(S, affine):
    key = (S, affine)
    if key not in _NC_CACHE:
        _NC_CACHE[key] = build(S, apply_ln_affine=affine)
    return _NC_CACHE[key]


def kernel(x, freqs, w_qkv, w_out, qn_w, qn_b, kn_w, kn_b):
    """Full-input entrypoint: shards across 8 neuron cores, runs, gathers."""
    import numpy as np
    from concourse.bass_utils import run_bass_kernel_spmd

    x = np.asarray(x, dtype=np.float32)
    freqs = np.asarray(freqs, dtype=np.float32)
    w_qkv = np.asarray(w_qkv, dtype=np.float32)
    w_out = np.asarray(w_out, dtype=np.float32)
    qn_w, qn_b = np.asarray(qn_w), np.asarray(qn_b)
    kn_w, kn_b = np.asarray(kn_w), np.asarray(kn_b)
    B, S, _ = x.shape
    affine = not (np.all(qn_w == 1) and np.all(qn_b == 0)
                  and np.all(kn_w == 1) and np.all(kn_b == 0))
    nc = _get_nc(S, bool(affine))
    maps = shard_inputs(x, freqs, w_qkv, w_out, qn_w, qn_b, kn_w, kn_b)
    res = run_bass_kernel_spmd(nc, maps, list(range(8)))
    return unshard_output(res.results, S)


# revision 6
# speedup vs baseline: 1.5111x; 1.2655x over previous
# Tensor-parallel fused attention kernel for TRN2, 8 cores — v3.
# Core r owns heads {2r, 2r+1}. Inputs per core:
#   x     [2*S, 1024] f32  (replicated; rows = b*S + s)
#   wqkv  [1024, 512] f32  (columns: q_h0|q_h1|k_h0|k_h1|v_h0|v_h1|g_h0|g_h1,
#                           64 each; q/k columns de-interleaved on host:
#                           new[i]=old[2i], new[32+i]=old[2i+1])
#   wout  [1024, 1024] f32 (full, replicated — out-projection is token-sliced)
#   freqs [S, 32] f32      (replicated)
# Output per core:
#   out   [2*TPC, 1024] f32  (rows: batch0 tokens r*TPC.., then batch1)
#
# v3 structure: software-pipelined batch-steps.  During attention of batch s,
# the qkv+LN+rope phase of batch s+1 fills PE/DVE/ACT gaps, and the
# out-projection of batch s-1 (token-sliced after an AllToAll) fills more.
# AllToAll moves 8x fewer bytes than the v1/v2 AllGather.  Softmax
# denominators ride the PV matmul as a ones-column on v.  Single ACT table
# set (natural_log_exp).  Rope on host-de-interleaved contiguous blocks.
import math

import concourse.bass as bass
import concourse.mybir as mybir
from concourse import bacc, tile

F32 = mybir.dt.float32
F16 = mybir.dt.float16
AF = mybir.ActivationFunctionType
ALU = mybir.AluOpType
AX = mybir.AxisListType

DIM = 1024
HD = 64
EPS = 1e-5


def build(S: int, n_cores: int = 8, reps: int = 1, apply_ln_affine: bool = False):
    TB = S // 128            # token tiles per batch
    QW = min(512, S)         # queries per attention tile
    QT = S // QW             # q tiles per batch
    NQ = QW // 128           # token tiles per q tile
    TPC = S // n_cores       # tokens per core per batch (out-proj slice)
    NSTEP = 2 * reps
    TPQ = (TB + QT - 1) // QT  # p1 tiles emitted per attention q-tile

    nc = bacc.Bacc("TRN2", target_bir_lowering=False, debug=False,
                   num_devices=n_cores)

    X = nc.dram_tensor("x", [2 * S, DIM], F32, kind="ExternalInput")
    WQKV = nc.dram_tensor("wqkv", [DIM, 512], F32, kind="ExternalInput")
    WOUT = nc.dram_tensor("wout", [DIM, DIM], F32, kind="ExternalInput")
    FREQS = nc.dram_tensor("freqs", [S, 32], F32, kind="ExternalInput")
    LNP = nc.dram_tensor("lnp", [8, HD], F32, kind="ExternalInput")
    OUT = nc.dram_tensor("out", [2 * TPC, DIM], F32, kind="ExternalOutput")

    ag_in = [nc.dram_tensor(f"ag_in{p}", [n_cores * 128, TPC], F16)
             for p in range(2)]
    ag_out = [nc.dram_tensor(f"ag_out{p}", [n_cores * 128, TPC], F16)
              for p in range(2)]

    with tile.TileContext(nc) as tc:
        with (
            tc.tile_pool(name="persist", bufs=1) as pp,
            tc.tile_pool(name="work", bufs=2) as wp,
            tc.tile_pool(name="espool", bufs=4) as ep,
            tc.tile_pool(name="xstage", bufs=3) as xp,
            tc.tile_pool(name="ogf", bufs=10) as fp,
            tc.tile_pool(name="small", bufs=2) as sp,
        ):
            # ---- constants & weights ----
            w16 = pp.tile([128, 8, 512], F16, tag="w16")
            w16of = pp.tile([128, 8, DIM], F16, tag="w16of")
            for c in range(8):
                w32 = wp.tile([128, 512], F32, tag="wld")
                nc.gpsimd.dma_start(w32[:], WQKV[c * 128:(c + 1) * 128, :])
                nc.vector.tensor_copy(w16[:, c, :], w32[:])
                for o in range(2):
                    wo32 = wp.tile([128, 512], F32, tag="wld")
                    nc.gpsimd.dma_start(
                        wo32[:], WOUT[c * 128:(c + 1) * 128,
                                      o * 512:(o + 1) * 512])
                    nc.vector.tensor_copy(w16of[:, c, o * 512:(o + 1) * 512],
                                          wo32[:])

            if apply_ln_affine:
                lnp1 = sp.tile([1, 512], F32, tag="lnp1")
                nc.gpsimd.dma_start(
                    lnp1[:], LNP.ap().rearrange("a b -> (a b)").unsqueeze(0))
                ones1 = sp.tile([1, 128], F32, tag="ones1")
                nc.vector.memset(ones1[:], 1.0)
                with tc.tile_pool(name="pbc", bufs=1, space="PSUM") as pbc:
                    lnb_ps = pbc.tile([128, 512], F32)
                    nc.tensor.matmul(lnb_ps[:], ones1[:], lnp1[:], start=True,
                                     stop=True)
                    lnwb = pp.tile([128, 512], F16, tag="lnwb")
                    nc.vector.tensor_copy(lnwb[:], lnb_ps[:])

            # cos/sin tables, duplicated across both 32-halves of each 64
            ftile = sp.tile([128, TB * 32], F32, tag="ftile")
            nc.gpsimd.dma_start(
                ftile[:].rearrange("p (a c) -> p a c", c=32),
                bass.AP(FREQS.ap().tensor, 0, [[32, 128], [128 * 32, TB], [1, 32]]),
            )
            sin32 = sp.tile([128, TB * 32], F32, tag="sin32")
            cos32 = sp.tile([128, TB * 32], F32, tag="cos32")
            halfpi = sp.tile([128, 1], F32, tag="halfpi")
            nc.vector.memset(halfpi[:], math.pi / 2)
            epsc = pp.tile([128, 1], F32, tag="epsc")
            nc.vector.memset(epsc[:], EPS)
            nc.scalar.activation(sin32[:], ftile[:], AF.Sin)
            nc.scalar.activation(cos32[:], ftile[:], AF.Sin, bias=halfpi[:])
            c2t = pp.tile([128, TB, 64], F16, tag="c2t")
            s2t = pp.tile([128, TB, 64], F16, tag="s2t")
            c32v = cos32[:].rearrange("p (a c) -> p a c", c=32)
            s32v = sin32[:].rearrange("p (a c) -> p a c", c=32)
            nc.vector.tensor_copy(c2t[:, :, 0:32], c32v)
            nc.vector.tensor_copy(c2t[:, :, 32:64], c32v)
            nc.vector.tensor_copy(s2t[:, :, 0:32], s32v)
            nc.vector.tensor_copy(s2t[:, :, 32:64], s32v)

            onesP = pp.tile([128, 64], F16, tag="onesP")
            nc.vector.memset(onesP[:], 1.0)
            rdf = pp.tile([128, 2 * QW], F16, tag="rdf")

            # ---- per-parity activation buffers (pipeline depth 2) ----
            xT_s = pp.tile([128, TB, 8, 128], F16, tag="xTs")
            xTall = [xT_s, xT_s]
            v16, qkgT, gh1, og0, og1 = [], [], [], [], []
            for p in range(2):
                v16_p = pp.tile([128, TB, 130], F16, tag=f"v16{p}")
                nc.vector.memset(v16_p[:, :, 64:65], 1.0)
                nc.vector.memset(v16_p[:, :, 129:130], 1.0)
                v16.append(v16_p)
                qkgT_p = pp.tile([128, TB, 3, 128], F16, tag=f"qkgT{p}")
                qkgT.append(qkgT_p)
                gh1_p = pp.tile([64, TB, 128], F16, tag=f"gh1{p}")
                gh1.append(gh1_p)
                og0_p = pp.tile([64, S], F16, tag=f"og0{p}")
                og0.append(og0_p)
                og1_p = pp.tile([64, S], F16, tag=f"og1{p}")
                og1.append(og1_p)

            # ---- shared staging (live only within one step) ----
            qkg16 = pp.tile([128, TB, 384], F16, tag="qkg16")
            xc16 = pp.tile([128, TB, 256], F16, tag="xc16")
            negm = pp.tile([128, TB * 4], F32, tag="negm")
            rstd = pp.tile([128, TB * 4], F32, tag="rstd")
            t16a = pp.tile([128, TB * 256], F16, tag="t16a")
            t16b = pp.tile([128, TB * 256], F16, tag="t16b")

            with (
                tc.tile_pool(name="ps_s", bufs=2, space="PSUM") as ps_s,
                tc.tile_pool(name="po0", bufs=1, space="PSUM") as pop0,
                tc.tile_pool(name="po1", bufs=1, space="PSUM") as pop1,
                tc.tile_pool(name="p1q", bufs=2, space="PSUM") as p1q,
            ):

                def x_stage(b, par, T0, T1):
                    """batched x load + transpose, 2 tiles per DMA pair."""
                    Tg = T0
                    while Tg < T1:
                        g = min(2, T1 - Tg)
                        xt16 = xp.tile([128, 2 * DIM], F16, tag="xt16")
                        xv = xt16[:, 0:g * DIM]
                        r0 = (b * TB + Tg) * 128
                        nc.gpsimd.dma_start(
                            xv.rearrange("p (j t) -> p j t", t=DIM),
                            bass.AP(X.ap().tensor, r0 * DIM,
                                    [[DIM, 128], [128 * DIM, g], [1, DIM]]))
                        nc.sync.dma_start_transpose(
                            xTall[par][:, Tg:Tg + g, :, :], xv)
                        Tg += g

                def p1_tile(b, par, T):
                    """qkv matmul + evac for one tile (x already staged)."""
                    psq = p1q.tile([128, 512], F32, tag="psq")
                    for c in range(8):
                        nc.tensor.matmul(psq[:], xTall[par][:, T, c, :],
                                         w16[:, c, :],
                                         start=(c == 0), stop=(c == 7))
                    nm = negm[:, T * 4:(T + 1) * 4]
                    qk_ps = psq[:, 0:256].rearrange("p (a b) -> p a b", b=HD)
                    nc.vector.tensor_reduce(nm, qk_ps, AX.X, ALU.add)
                    nc.vector.tensor_scalar_mul(nm, nm, -1.0 / HD)
                    nc.vector.tensor_tensor(
                        xc16[:, T, :].rearrange("p (a b) -> p a b", b=HD),
                        qk_ps,
                        nm.unsqueeze(2).broadcast_to([128, 4, HD]), ALU.add)
                    v = v16[par]
                    nc.vector.tensor_copy(
                        bass.AP(v.tensor, v[:, T, 0].offset,
                                [[v[:].ap[0][0], 128], [65, 2], [1, HD]]),
                        psq[:, 256:384].rearrange("p (a b) -> p a b", b=HD))
                    nc.scalar.activation(qkg16[:, T, 256:384], psq[:, 384:512],
                                         AF.Exp, scale=-1.0)

                def p1_batch_range(b, par, T0, T1):
                    """LN rstd + scale + rope + sigmoid + transposes."""
                    H = T1 - T0
                    xch = xc16[:, T0:T0 + H, :]
                    xcf = xch.rearrange("p a b -> p (a b)")
                    th_a = t16a[:, T0 * 256:(T0 + H) * 256]
                    nc.vector.tensor_tensor(th_a, xcf, xcf, ALU.mult)
                    ssq = rstd[:, T0 * 4:(T0 + H) * 4]
                    nc.vector.tensor_reduce(
                        ssq, th_a.rearrange("p (a b) -> p a b", b=HD),
                        AX.X, ALU.add)
                    nc.scalar.activation(ssq, ssq, AF.Ln, bias=epsc[:],
                                         scale=1.0 / HD)
                    nc.scalar.activation(ssq, ssq, AF.Exp, scale=-0.5)
                    xcv = xch.rearrange("p a (s d) -> p (a s) d", d=HD)
                    nc.vector.tensor_tensor(
                        xcv, xcv,
                        ssq.unsqueeze(2).broadcast_to([128, H * 4, HD]),
                        ALU.mult)
                    if apply_ln_affine:
                        nc.vector.tensor_tensor(
                            xch, xch,
                            lnwb[:, 0:256].unsqueeze(1).broadcast_to(
                                [128, H, 256]), ALU.mult)
                        nc.vector.tensor_tensor(
                            xch, xch,
                            lnwb[:, 256:512].unsqueeze(1).broadcast_to(
                                [128, H, 256]), ALU.add)
                    # rope: ta = x'*c2, tb = x'*s2;
                    # r1 = ta[0:32]-tb[32:64]; r2 = tb[0:32]+ta[32:64]
                    xc4 = xch.rearrange("p a (s d) -> p a s d", d=HD)
                    cstep = c2t[:].ap[0][0]
                    c2b = bass.AP(c2t.tensor, c2t[:, T0, 0].offset,
                                  [[cstep, 128], [64, H], [0, 4], [1, 64]])
                    s2b = bass.AP(s2t.tensor, s2t[:, T0, 0].offset,
                                  [[cstep, 128], [64, H], [0, 4], [1, 64]])
                    ta = t16a[:, T0 * 256:(T0 + H) * 256].rearrange(
                        "p (a s d) -> p a s d", s=4, d=HD)
                    tb = t16b[:, T0 * 256:(T0 + H) * 256].rearrange(
                        "p (a s d) -> p a s d", s=4, d=HD)
                    nc.vector.tensor_tensor(ta, xc4, c2b, ALU.mult)
                    nc.vector.tensor_tensor(tb, xc4, s2b, ALU.mult)
                    qkv4 = qkg16[:, T0:T0 + H, 0:256].rearrange(
                        "p a (s d) -> p a s d", d=HD)
                    nc.vector.tensor_tensor(
                        qkv4[:, :, :, 0:32], ta[:, :, :, 0:32],
                        tb[:, :, :, 32:64], ALU.subtract)
                    nc.vector.tensor_tensor(
                        qkv4[:, :, :, 32:64], tb[:, :, :, 0:32],
                        ta[:, :, :, 32:64], ALU.add)
                    # sigmoid = 1/(1+exp(-g)); exp(-g) already staged
                    gview = qkg16[:, T0:T0 + H, 256:384]
                    tg = t16b[:, T0 * 128:(T0 + H) * 128].rearrange(
                        "p (a c) -> p a c", c=128)
                    nc.vector.tensor_scalar_add(tg, gview, 1.0)
                    with nc.allow_low_precision(reason="f16 sigmoid ok"):
                        nc.vector.reciprocal(gview, tg)
                    Tg = T0
                    while Tg < T0 + H:
                        g = min(4, T0 + H - Tg)
                        nc.sync.dma_start_transpose(
                            qkgT[par][:, Tg:Tg + g, :, :],
                            qkg16[:, Tg:Tg + g, :])
                        Tg += g
                    # shift gate h1 (partitions 64:128) down to 0:64
                    nc.gpsimd.dma_start(gh1[par][:, T0:T0 + H, :],
                                        qkgT[par][64:128, T0:T0 + H, 2, :])

                def attn_jloop(b, par, Q, po01):
                    qs0 = qkgT[par][0:64, Q * NQ:(Q + 1) * NQ, 0, :]
                    qs1 = qkgT[par][64:128, Q * NQ:(Q + 1) * NQ, 0, :]
                    po0, po1 = po01
                    for J in range(TB):
                        ps = ps_s.tile([128, 2 * QW], F32, tag="ps")
                        nc.tensor.matmul(ps[:, 0:QW], qkgT[par][0:64, J, 1, :],
                                         qs0, start=True, stop=True)
                        nc.tensor.matmul(ps[:, QW:2 * QW],
                                         qkgT[par][64:128, J, 1, :],
                                         qs1, start=True, stop=True)
                        es = ep.tile([128, 2 * QW], F16, tag="es")
                        nc.scalar.activation(es[:], ps[:], AF.Exp, scale=0.125)
                        nc.tensor.matmul(po0[:], v16[par][:, J, 0:65],
                                         es[:, 0:QW], start=(J == 0),
                                         stop=(J == TB - 1),
                                         skip_group_check=True)
                        nc.tensor.matmul(po1[:], v16[par][:, J, 65:130],
                                         es[:, QW:2 * QW], start=(J == 0),
                                         stop=(J == TB - 1),
                                         skip_group_check=True)

                def attn_finish(b, par, Q, po01):
                    po0, po1 = po01
                    with nc.allow_low_precision(reason="f16 recip-den"):
                        nc.vector.reciprocal(rdf[64:65, 0:QW], po0[64:65, :])
                        nc.vector.reciprocal(rdf[64:65, QW:2 * QW],
                                             po1[64:65, :])
                    prt = ps_s.tile([128, 2 * QW], F32, tag="ps")
                    pr0 = prt[0:64, 0:QW]
                    pr1 = prt[0:64, QW:2 * QW]
                    nc.tensor.matmul(pr0, onesP[64:65, :], rdf[64:65, 0:QW],
                                     start=True, stop=True)
                    nc.tensor.matmul(pr1, onesP[64:65, :],
                                     rdf[64:65, QW:2 * QW],
                                     start=True, stop=True)
                    gq0 = qkgT[par][0:64, Q * NQ:(Q + 1) * NQ, 2, :]
                    gq1 = gh1[par][:, Q * NQ:(Q + 1) * NQ, :]
                    u0 = wp.tile([64, QW], F32, tag="u")
                    u1 = wp.tile([64, QW], F32, tag="u")
                    nc.vector.tensor_tensor(
                        u0[:].rearrange("p (a t) -> p a t", t=128),
                        pr0.rearrange("p (a t) -> p a t", t=128), gq0, ALU.mult)
                    nc.vector.tensor_tensor(
                        u1[:].rearrange("p (a t) -> p a t", t=128),
                        pr1.rearrange("p (a t) -> p a t", t=128), gq1, ALU.mult)
                    qoff = Q * QW
                    nc.vector.tensor_tensor(og0[par][:, qoff:qoff + QW],
                                            po0[0:64, :], u0[:], ALU.mult)
                    nc.vector.tensor_tensor(og1[par][:, qoff:qoff + QW],
                                            po1[0:64, :], u1[:], ALU.mult)

                def attn_q(b, par, Q):
                    qs0 = qkgT[par][0:64, Q * NQ:(Q + 1) * NQ, 0, :]
                    qs1 = qkgT[par][64:128, Q * NQ:(Q + 1) * NQ, 0, :]
                    po0 = pop0.tile([65, QW], F32, tag="po0")
                    po1 = pop1.tile([65, QW], F32, tag="po1")
                    for J in range(TB):
                        ps = ps_s.tile([128, 2 * QW], F32, tag="ps")
                        nc.tensor.matmul(ps[:, 0:QW], qkgT[par][0:64, J, 1, :],
                                         qs0, start=True, stop=True)
                        nc.tensor.matmul(ps[:, QW:2 * QW],
                                         qkgT[par][64:128, J, 1, :],
                                         qs1, start=True, stop=True)
                        es = ep.tile([128, 2 * QW], F16, tag="es")
                        nc.scalar.activation(es[:], ps[:], AF.Exp, scale=0.125)
                        nc.tensor.matmul(po0[:], v16[par][:, J, 0:65],
                                         es[:, 0:QW], start=(J == 0),
                                         stop=(J == TB - 1),
                                         skip_group_check=True)
                        nc.tensor.matmul(po1[:], v16[par][:, J, 65:130],
                                         es[:, QW:2 * QW], start=(J == 0),
                                         stop=(J == TB - 1),
                                         skip_group_check=True)
                    with nc.allow_low_precision(reason="f16 recip-den"):
                        nc.vector.reciprocal(rdf[64:65, 0:QW], po0[64:65, :])
                        nc.vector.reciprocal(rdf[64:65, QW:2 * QW],
                                             po1[64:65, :])
                    prt = ps_s.tile([128, 2 * QW], F32, tag="ps")
                    pr0 = prt[0:64, 0:QW]
                    pr1 = prt[0:64, QW:2 * QW]
                    nc.tensor.matmul(pr0, onesP[64:65, :], rdf[64:65, 0:QW],
                                     start=True, stop=True)
                    nc.tensor.matmul(pr1, onesP[64:65, :],
                                     rdf[64:65, QW:2 * QW],
                                     start=True, stop=True)
                    gq0 = qkgT[par][0:64, Q * NQ:(Q + 1) * NQ, 2, :]
                    gq1 = gh1[par][:, Q * NQ:(Q + 1) * NQ, :]
                    u0 = wp.tile([64, QW], F32, tag="u")
                    u1 = wp.tile([64, QW], F32, tag="u")
                    nc.vector.tensor_tensor(
                        u0[:].rearrange("p (a t) -> p a t", t=128),
                        pr0.rearrange("p (a t) -> p a t", t=128), gq0, ALU.mult)
                    nc.vector.tensor_tensor(
                        u1[:].rearrange("p (a t) -> p a t", t=128),
                        pr1.rearrange("p (a t) -> p a t", t=128), gq1, ALU.mult)
                    qoff = Q * QW
                    nc.vector.tensor_tensor(og0[par][:, qoff:qoff + QW],
                                            po0[0:64, :], u0[:], ALU.mult)
                    nc.vector.tensor_tensor(og1[par][:, qoff:qoff + QW],
                                            po1[0:64, :], u1[:], ALU.mult)

                def emit_a2a(par):
                    # ag_in rows j*128+p <- og{0,1}[p, j*TPC + t]
                    ogt = og0[par]
                    nc.gpsimd.dma_start(
                        bass.AP(ag_in[par].ap().tensor, 0,
                                [[TPC, 64], [128 * TPC, n_cores], [1, TPC]]),
                        ogt[:].rearrange("p (j t) -> p j t", t=TPC))
                    ogt = og1[par]
                    nc.gpsimd.dma_start(
                        bass.AP(ag_in[par].ap().tensor, 64 * TPC,
                                [[TPC, 64], [128 * TPC, n_cores], [1, TPC]]),
                        ogt[:].rearrange("p (j t) -> p j t", t=TPC))
                    nc.gpsimd.collective_compute(
                        "AllToAll", ALU.bypass,
                        replica_groups=[list(range(n_cores))],
                        ins=[ag_in[par].ap()], outs=[ag_out[par].ap()],
                    )

                def outproj_load(par):
                    tiles = []
                    for c in range(8):
                        ogf = fp.tile([128, TPC], F16, tag="ogf")
                        nc.sync.dma_start(
                            ogf[:], ag_out[par][c * 128:(c + 1) * 128, :])
                        tiles.append(ogf)
                    return tiles

                TW = min(128, TPC)       # tokens per out-proj tile
                NOT = TPC // TW          # out-proj tiles per step

                def outproj_mm(b, tiles, i):
                    # token tile i (TW tokens) -> OUT rows b*TPC + i*TW
                    for o in range(DIM // 512):
                        pot = p1q.tile([128, 512], F32, tag="psq")
                        potv = pot[0:TW, :]
                        for c in range(8):
                            nc.tensor.matmul(
                                potv, tiles[c][:, i * TW:(i + 1) * TW],
                                w16of[:, c, o * 512:(o + 1) * 512],
                                start=(c == 0), stop=(c == 7))
                        ot32 = wp.tile([128, 512], F32, tag="ot32")
                        nc.vector.tensor_copy(ot32[0:TW, :], potv)
                        nc.gpsimd.dma_start(
                            OUT[b * TPC + i * TW: b * TPC + (i + 1) * TW,
                                o * 512:(o + 1) * 512], ot32[0:TW, :])

                # ==== prologue: full P1 for batch-step 0 ====
                x_stage(0, 0, 0, TB)
                for T in range(TB):
                    p1_tile(0, 0, T)
                p1_batch_range(0, 0, 0, TB // 2)
                p1_batch_range(0, 0, TB // 2, TB)

                # ==== steady-state steps ====
                prev_tiles = None
                for s in range(NSTEP):
                    b, par = s % 2, s % 2
                    nb, npar = (s + 1) % 2, (s + 1) % 2
                    has_next = s + 1 < NSTEP
                    # transposes serialize with in-flight collectives, so
                    # none of batch s+1's transposes may be scheduled while
                    # A2A(s-1) runs (the first ~30us of this step): p1 tiles
                    # go at the end of Q1 and right before the last q-tile.
                    t1 = 5 * TB // 8
                    for Q in range(QT):
                        po0_t = pop0.tile([65, QW], F32, tag="po0")
                        po1_t = pop1.tile([65, QW], F32, tag="po1")
                        po01 = (po0_t, po1_t)
                        attn_jloop(b, par, Q, po01)
                        if has_next and QT >= 3:
                            if Q == 1:
                                x_stage(nb, npar, 0, t1)
                                for T in range(0, t1):
                                    p1_tile(nb, npar, T)
                            elif Q == QT - 2:
                                x_stage(nb, npar, t1, TB)
                                for T in range(t1, TB):
                                    p1_tile(nb, npar, T)
                        attn_finish(b, par, Q, po01)
                        if has_next and QT >= 3:
                            if Q == 1:
                                p1_batch_range(nb, npar, 0, t1)
                            elif Q == QT - 2:
                                p1_batch_range(nb, npar, t1, TB)
                        if s >= 1 and QT >= 3 and Q == QT - 2:
                            prev_tiles = outproj_load(1 - par)
                    if has_next and QT < 3:
                        x_stage(nb, npar, 0, TB)
                        for T in range(TB):
                            p1_tile(nb, npar, T)
                        p1_batch_range(nb, npar, 0, TB)
                    if s >= 1 and QT < 3:
                        prev_tiles = outproj_load(1 - par)
                    if s >= 1:
                        for i in range(NOT):
                            outproj_mm(1 - b, prev_tiles, i)
                    emit_a2a(par)

                # ==== epilogue: out-projection of the last step ====
                last_par = (NSTEP - 1) % 2
                tiles = outproj_load(last_par)
                for i in range(NOT):
                    outproj_mm((NSTEP - 1) % 2, tiles, i)

    nc.compile()
    return nc


def _deinterleave_cols(w):
    # [..., 64] -> new[i]=old[2i], new[32+i]=old[2i+1]
    import numpy as np
    return np.concatenate([w[..., 0::2], w[..., 1::2]], axis=-1)


def shard_inputs(x, freqs, w_qkv, w_out, qn_w, qn_b, kn_w, kn_b, n_cores=8):
    import numpy as np
    B, S, _ = x.shape
    x2 = np.ascontiguousarray(x.reshape(2 * S, DIM), dtype=np.float32)
    qw = _deinterleave_cols(np.asarray(qn_w, np.float32))
    qb = _deinterleave_cols(np.asarray(qn_b, np.float32))
    kw = _deinterleave_cols(np.asarray(kn_w, np.float32))
    kb = _deinterleave_cols(np.asarray(kn_b, np.float32))
    lnp = np.stack([qw, qw, kw, kw, qb, qb, kb, kb]).astype(np.float32)
    wof = np.ascontiguousarray(w_out, dtype=np.float32)
    maps = []
    for r in range(n_cores):
        cols = []
        for sec in range(4):
            for h in range(2):
                c0 = sec * DIM + HD * (2 * r + h)
                blk = w_qkv[:, c0:c0 + HD]
                if sec < 2:  # q, k: de-interleave feature columns
                    blk = _deinterleave_cols(blk)
                cols.append(blk)
        wq = np.ascontiguousarray(np.concatenate(cols, axis=1), dtype=np.float32)
        maps.append({
            "x": x2, "wqkv": wq, "wout": wof,
            "freqs": np.ascontiguousarray(freqs, dtype=np.float32),
            "lnp": lnp,
        })
    return maps


def unshard_output(results, S):
    import numpy as np
    n = len(results)
    TPC = S // n
    out = np.empty((2, S, DIM), np.float32)
    for r in range(n):
        o = results[r]["out"]
        out[0, r * TPC:(r + 1) * TPC] = o[0:TPC]
        out[1, r * TPC:(r + 1) * TPC] = o[TPC:2 * TPC]
    return out


_NC_CACHE = {}


def _get_nc(S, affine):
    key = (S, affine)
    if key not in _NC_CACHE:
        _NC_CACHE[key] = build(S, apply_ln_affine=affine)
    return _NC_CACHE[key]


def kernel(x, freqs, w_qkv, w_out, qn_w, qn_b, kn_w, kn_b):
    """Full-input entrypoint: shards across 8 neuron cores, runs, gathers."""
    import numpy as np
    from concourse.bass_utils import run_bass_kernel_spmd

    x = np.asarray(x, dtype=np.float32)
    freqs = np.asarray(freqs, dtype=np.float32)
    w_qkv = np.asarray(w_qkv, dtype=np.float32)
    w_out = np.asarray(w_out, dtype=np.float32)
    qn_w, qn_b = np.asarray(qn_w), np.asarray(qn_b)
    kn_w, kn_b = np.asarray(kn_w), np.asarray(kn_b)
    B, S, _ = x.shape
    affine = not (np.all(qn_w == 1) and np.all(qn_b == 0)
                  and np.all(kn_w == 1) and np.all(kn_b == 0))
    nc = _get_nc(S, bool(affine))
    maps = shard_inputs(x, freqs, w_qkv, w_out, qn_w, qn_b, kn_w, kn_b)
    res = run_bass_kernel_spmd(nc, maps, list(range(8)))
    return unshard_output(res.results, S)
